# revision 1
# baseline (speedup 1.0000x reference)
"""ArgumentGCN message-passing kernel for TRN2, data-parallel over batch on 8 cores.

v3: mixed precision. The self/dw path (whose error hits the output directly)
stays bf16; the projection+aggregation path (whose error is damped by the
1/nn_num averaging) runs fp8 with DoubleRow (2 contraction rows per PE cell).

Per core (BL=4 local batches, N=512 nodes, D=256 feats, K=6 edge types, 2 steps),
all in transposed orientation (features/sources on partitions):

  dw[n]   = sigmoid(sum_d nodeT[d,n] * w_nw[d] + b_nw)              (bf16)
  P'[n,:] = (dw[n]/16) * sum_d nodeT_f8[d,n] * (16*W_k[d,e])        (fp8 DR)
  A[e,m]  = sum_{k,n} P'[n,k*256+e] * Gt[k][n,m]                    (fp8 DR)
  S[e,m]  = sum_d w_self[d,e] * nodeT[d,m]                          (bf16)
  new[e,m]= relu(A[e,m]*inv_nn[m] + S[e,m] + b_self[e])

W_k are host-scaled by 16 so ~0.02-magnitude entries stay fp8-normal; the 1/16
rides on the eviction scalar. Graphs are exact {0,1} fp8 (dd-masked, diag
zeroed on host); 1/nn_num is applied at eviction via a per-batch broadcast
tile built once with a K=1 matmul.
"""

import sys

for _p in ("/opt/trn_rl_repo",):
    if _p not in sys.path:
        sys.path.insert(0, _p)

import numpy as np
import ml_dtypes

import concourse.bass as bass
import concourse.mybir as mybir
import concourse.tile as tile
from concourse import bacc
from concourse.bass_utils import run_bass_kernel_spmd

B, N, D = 32, 512, 256
KEDGE = 6
STEPS = 2
NCORES = 8
BL = B // NCORES          # batches per core
NSLAB = N // 128          # 4 n-slabs
DSLAB = D // 128          # 2 d-slabs
ECH = D // 128            # 2 e-chunks
WECOLS = KEDGE * D        # 1536
PCHUNK = 512              # matmul free-dim / PSUM bank size (f32)
WSCALE = 16.0             # host pre-scale on edge weights (fp8-normal range)

F32 = mybir.dt.float32
BF16 = mybir.dt.bfloat16
FP8 = mybir.dt.float8e4
DR = mybir.MatmulPerfMode.DoubleRow
NPFP8 = ml_dtypes.float8_e4m3
NPBF16 = ml_dtypes.bfloat16


def build_nc(interleave=False, nall=True, psmain=5, psagg=3, ppbufs=6, tubufs=4):
    nc = bacc.Bacc("TRN2", target_bir_lowering=False, debug=False, num_devices=NCORES)

    # inputs (per-core shards; layouts chosen for DMA efficiency)
    gt = nc.dram_tensor("gt", [KEDGE, BL, 128, NSLAB, N], FP8, kind="ExternalInput")
    node0 = nc.dram_tensor("node0", [BL, DSLAB, 128, N], BF16, kind="ExternalInput")
    node0f = nc.dram_tensor("node0f", [BL, DSLAB, 128, N], FP8, kind="ExternalInput")
    we = nc.dram_tensor("we", [DSLAB, 128, WECOLS], FP8, kind="ExternalInput")
    ws = nc.dram_tensor("ws", [DSLAB, 128, D], BF16, kind="ExternalInput")
    wnw = nc.dram_tensor("wnw", [DSLAB, 128, 1], BF16, kind="ExternalInput")
    bself = nc.dram_tensor("bself", [ECH, 128, 1], F32, kind="ExternalInput")
    bnw = nc.dram_tensor("bnw", [128, 1], F32, kind="ExternalInput")
    invnn = nc.dram_tensor("invnn", [1, BL * N], F32, kind="ExternalInput")

    # outputs
    node_out = nc.dram_tensor("node_out", [BL, ECH, 128, N], F32, kind="ExternalOutput")
    w_out = nc.dram_tensor("w_out", [STEPS, BL, NSLAB, 128, 1], F32, kind="ExternalOutput")

    with tile.TileContext(nc) as tc:
        with (
            tc.tile_pool(name="consts", bufs=1) as consts,
            tc.tile_pool(name="gtp", bufs=KEDGE * BL) as gtp,
            tc.tile_pool(name="nodep", bufs=BL * STEPS) as nodep,
            tc.tile_pool(name="nodefp", bufs=BL * STEPS) as nodefp,
            tc.tile_pool(name="pprime", bufs=ppbufs) as pprime,
            tc.tile_pool(name="dwp", bufs=4) as dwp,
            tc.tile_pool(name="tup", bufs=tubufs) as tup,
            tc.tile_pool(name="foutp", bufs=2) as foutp,
            tc.tile_pool(name="ps_main", bufs=psmain, space="PSUM") as ps_main,
            tc.tile_pool(name="ps_agg", bufs=psagg, space="PSUM") as ps_agg,
        ):
            # ---- scalar HWDGE ring: we first, then batch-0 graphs ----
            we_sb = consts.tile([128, DSLAB, WECOLS], FP8)
            nc.scalar.dma_start(out=we_sb[:], in_=we.ap().rearrange("d p c -> p d c"))
            gt_sb = {}
            for k in range(KEDGE):
                t = gtp.tile([128, NSLAB, N], FP8, tag="gt", name="gt_sb")
                nc.scalar.dma_start(out=t[:], in_=gt.ap()[k, 0])
                gt_sb[(k, 0)] = t

            # ---- sync queue: invnn first (gates the DVE stream via invB), then
            # batch-0 state + weights so PE starts asap ----
            invrow = consts.tile([1, BL * N], F32)
            nc.sync.dma_start(out=invrow[:], in_=invnn.ap())
            node_sb, nodef_sb = {}, {}

            def load_node(b):
                t = nodep.tile([128, DSLAB, N], BF16, tag="node", name="node_in")
                nc.sync.dma_start(out=t[:], in_=node0.ap()[b].rearrange("d p m -> p d m"))
                node_sb[b] = t
                tf = nodefp.tile([128, DSLAB, N], FP8, tag="nodef", name="nodef_in")
                nc.sync.dma_start(out=tf[:], in_=node0f.ap()[b].rearrange("d p m -> p d m"))
                nodef_sb[b] = tf

            load_node(0)
            wnw_sb = consts.tile([128, DSLAB, 1], BF16)
            nc.sync.dma_start(out=wnw_sb[:], in_=wnw.ap().rearrange("d p c -> p d c"))
            bnw_sb = consts.tile([128, 1], F32)
            nc.sync.dma_start(out=bnw_sb[:], in_=bnw.ap())
            if nall:
                # batches 1..3 initial state in two consolidated DMAs
                nt = consts.tile([128, BL - 1, DSLAB, N], BF16, name="nall")
                nc.sync.dma_start(
                    out=nt[:], in_=node0.ap()[1:BL].rearrange("b d p m -> p b d m")
                )
                ntf = consts.tile([128, BL - 1, DSLAB, N], FP8, name="nallf")
                nc.sync.dma_start(
                    out=ntf[:], in_=node0f.ap()[1:BL].rearrange("b d p m -> p b d m")
                )
                for b in range(1, BL):
                    node_sb[b] = nt[:, b - 1]
                    nodef_sb[b] = ntf[:, b - 1]
            else:
                for b in range(1, BL):
                    load_node(b)
            ws_sb = consts.tile([128, DSLAB, D], BF16)
            nc.sync.dma_start(out=ws_sb[:], in_=ws.ap().rearrange("d p c -> p d c"))
            bself_sb = consts.tile([128, ECH, 1], F32)
            nc.sync.dma_start(out=bself_sb[:], in_=bself.ap().rearrange("c p o -> p c o"))
            ones_sb = consts.tile([1, 128], F32)
            nc.vector.memset(ones_sb[:], 1.0)

            # ---- per-batch 1/nn broadcast tiles (K=1 matmul broadcast);
            # early: the DVE copies gate all later DVE evictions ----
            invB = {}
            for b in range(BL):
                pib = ps_main.tile([128, PCHUNK], F32, tag="ps", name="ps_invb")
                nc.tensor.matmul(
                    pib[:], lhsT=ones_sb[:], rhs=invrow[:, b * N : (b + 1) * N],
                    start=True, stop=True,
                )
                t = consts.tile([128, N], F32, tag="invb", name="invb", bufs=BL)
                nc.vector.tensor_copy(t[:], pib[:])
                invB[b] = t

            # ---- remaining graphs on the sync queue, batch-major ----
            for b in range(1, BL):
                for k in range(KEDGE):
                    t = gtp.tile([128, NSLAB, N], FP8, tag="gt", name="gt_sb")
                    nc.sync.dma_start(out=t[:], in_=gt.ap()[k, b])
                    gt_sb[(k, b)] = t

            # ---- iteration steps ----
            pp_sb, dwcs_sb = {}, {}

            def emit_dw_proj(step, b):
                    cur = node_sb[b]
                    curf = nodef_sb[b]

                    # (1) node-relatedness weights, column form per n-slab (bf16)
                    dwps = ps_main.tile([128, NSLAB], F32, tag="ps", name="ps_dw")
                    for s in range(NSLAB):
                        for d in range(DSLAB):
                            nc.tensor.matmul(
                                dwps[:, s : s + 1],
                                lhsT=cur[:, d, s * 128 : (s + 1) * 128],
                                rhs=wnw_sb[:, d, :],
                                start=(d == 0),
                                stop=(d == DSLAB - 1),
                            )
                    dwcol = dwp.tile([128, NSLAB], F32, tag="dwcol", name="dwcol")
                    nc.scalar.activation(
                        dwcol[:], dwps[:],
                        mybir.ActivationFunctionType.Sigmoid,
                        bias=bnw_sb[:],
                    )
                    for s in range(NSLAB):
                        nc.sync.dma_start(
                            out=w_out.ap()[step, b, s], in_=dwcol[:, s : s + 1]
                        )
                    dwcs = dwp.tile([128, NSLAB], F32, tag="dwcs", name="dwcs")
                    nc.vector.tensor_scalar_mul(dwcs[:], dwcol[:], 1.0 / WSCALE)
                    dwcs_sb[b] = dwcs

                    # (2) projections (fp8 DoubleRow over d), scaled at eviction
                    pp = pprime.tile([128, NSLAB, WECOLS], FP8, tag="pp", name="pp")
                    pp_sb[b] = pp
                    for s in range(NSLAB):
                        for c in range(WECOLS // PCHUNK):
                            pch = ps_main.tile([128, PCHUNK], F32, tag="ps", name="ps_proj")
                            nc.tensor.matmul(
                                pch[:],
                                lhsT=curf[:, 0:DSLAB, s * 128 : (s + 1) * 128],
                                rhs=we_sb[:, 0:DSLAB, c * PCHUNK : (c + 1) * PCHUNK],
                                start=True, stop=True, perf_mode=DR,
                            )
                            dst = pp[:, s, c * PCHUNK : (c + 1) * PCHUNK]
                            if c % 2 == 0:
                                nc.vector.tensor_scalar_mul(dst, pch[:], dwcs[:, s : s + 1])
                            else:
                                nc.scalar.mul(dst, pch[:], dwcs[:, s : s + 1])

            def emit_agg(step, b):
                    last = step == STEPS - 1
                    cur = node_sb[b]
                    pp = pp_sb[b]
                    # (3) self (bf16) + aggregation (fp8 DR), inv_nn at eviction
                    if last:
                        out_t = foutp.tile([128, ECH, N], F32, tag="fout", name="fout")
                    else:
                        out_t = nodep.tile([128, DSLAB, N], BF16, tag="node", name="node_nx")
                        outf_t = nodefp.tile([128, DSLAB, N], FP8, tag="nodef", name="nodef_nx")
                    for c in range(ECH):
                        pa = ps_agg.tile([128, N], F32, tag="psagg", name="ps_agg")
                        ps_self = ps_agg.tile([128, N], F32, tag="psagg", name="ps_self")
                        for d in range(DSLAB):
                            nc.tensor.matmul(
                                ps_self[:],
                                lhsT=ws_sb[:, d, c * 128 : (c + 1) * 128],
                                rhs=cur[:, d, :],
                                start=(d == 0), stop=(d == DSLAB - 1),
                            )
                        for sp in range(NSLAB // 2):
                            for k in range(KEDGE):
                                nc.tensor.matmul(
                                    pa[:],
                                    lhsT=pp[:, 2 * sp : 2 * sp + 2,
                                            k * D + c * 128 : k * D + (c + 1) * 128],
                                    rhs=gt_sb[(k, b)][:, 2 * sp : 2 * sp + 2, :],
                                    start=(sp == 0 and k == 0),
                                    stop=(sp == NSLAB // 2 - 1 and k == KEDGE - 1),
                                    perf_mode=DR,
                                )
                        t_t = tup.tile([128, N], F32, tag="tu", name="t_t")
                        nc.vector.tensor_tensor(
                            out=t_t[:], in0=pa[:], in1=invB[b][:],
                            op=mybir.AluOpType.mult,
                        )
                        u_t = tup.tile([128, N], F32, tag="tu", name="u_t")
                        nc.vector.scalar_tensor_tensor(
                            out=u_t[:], in0=ps_self[:], scalar=1.0,
                            in1=t_t[:],
                            op0=mybir.AluOpType.mult, op1=mybir.AluOpType.add,
                        )
                        nc.scalar.activation(
                            out_t[:, c, :], u_t[:],
                            mybir.ActivationFunctionType.Relu,
                            bias=bself_sb[:, c, :],
                        )
                        if last:
                            nc.sync.dma_start(
                                out=node_out.ap()[b, c], in_=out_t[:, c, :]
                            )
                        else:
                            nc.vector.tensor_copy(outf_t[:, c, :], out_t[:, c, :])
                    if last:
                        pass
                    else:
                        node_sb[b] = out_t
                        nodef_sb[b] = outf_t

            # step 0: front-load every batch's dw/proj before the first agg so
            # PE has work while the graphs stream in; then software-pipeline:
            # step-1 dw/proj for batch b emits right after step-0's agg of b.
            for b in range(BL):
                emit_dw_proj(0, b)
            if interleave:
                emit_agg(0, 0)
                emit_agg(0, 1)
                emit_dw_proj(1, 0)
                emit_agg(0, 2)
                emit_dw_proj(1, 1)
                emit_agg(0, 3)
                emit_dw_proj(1, 2)
                emit_agg(1, 0)
                emit_dw_proj(1, 3)
                emit_agg(1, 1)
                emit_agg(1, 2)
                emit_agg(1, 3)
            else:
                for b in range(BL):
                    emit_agg(0, b)
                for b in range(BL):
                    emit_dw_proj(1, b)
                    emit_agg(1, b)

    nc.compile()
    return nc


_NC_CACHE = None


def get_nc():
    global _NC_CACHE
    if _NC_CACHE is None:
        _NC_CACHE = build_nc()
    return _NC_CACHE


def _fp8(x):
    return np.clip(np.asarray(x, np.float32), -240.0, 240.0).astype(NPFP8)


def _prep_core_inputs(node, node_mask, graphs, params):
    """Host-side shard + layout prep. Returns in_maps list (one dict per core)."""
    f32 = np.float32
    mask = node_mask.astype(f32)                          # [B,N]
    dd = mask[:, None, :] * mask[:, :, None]              # [B,N,N]
    dd[:, np.arange(N), np.arange(N)] = 0.0

    G = np.stack(graphs, 0).astype(f32) * dd[None]        # [K,B,N,N]
    nn = G.sum(axis=(0, -1))                              # [B,N] dest counts
    nn = np.where(nn >= 1.0, nn, 1.0)
    inv_nn = (1.0 / nn).astype(f32)                       # [B,N]
    # transpose to [K,B,n,m], layout [K,B,part,slab,m] with n = slab*128+part
    Gt = np.ascontiguousarray(G.transpose(0, 1, 3, 2))
    Gt = Gt.reshape(KEDGE, B, NSLAB, 128, N).transpose(0, 1, 3, 2, 4)
    Gt = np.ascontiguousarray(Gt).astype(NPFP8)           # exact {0,1}

    nodeT = np.ascontiguousarray(node.transpose(0, 2, 1)) # [B,D,N]
    nodeT = nodeT.reshape(B, DSLAB, 128, N).astype(NPBF16)
    nodeTf = _fp8(nodeT.astype(f32))

    we = _fp8(WSCALE * np.concatenate(
        [params[k] for k in ("w_arg1", "w_arg2", "w_arg3", "w_arg4",
                             "w_punct", "w_punct_re")], axis=1
    ).reshape(DSLAB, 128, WECOLS))
    ws = params["w_self"].reshape(DSLAB, 128, D).astype(NPBF16)
    wnw = params["w_nw"].reshape(DSLAB, 128, 1).astype(NPBF16)
    bself = params["b_self"].astype(f32).reshape(ECH, 128, 1)
    bnw = np.full((128, 1), np.float32(params["b_nw"][0]), dtype=f32)

    in_maps = []
    for core in range(NCORES):
        sl = slice(core * BL, (core + 1) * BL)
        in_maps.append({
            "gt": np.ascontiguousarray(Gt[:, sl]),
            "node0": np.ascontiguousarray(nodeT[sl]),
            "node0f": np.ascontiguousarray(nodeTf[sl]),
            "invnn": np.ascontiguousarray(inv_nn[sl]).reshape(1, BL * N),
            "we": we, "ws": ws, "wnw": wnw, "bself": bself, "bnw": bnw,
        })
    return in_maps


def run(node, node_mask, graphs, params, trace=False, **spmd_kwargs):
    nc = get_nc()
    in_maps = _prep_core_inputs(node, node_mask, graphs, params)
    res = run_bass_kernel_spmd(
        nc, in_maps, core_ids=list(range(NCORES)), trace=trace, **spmd_kwargs
    )
    node_parts, w_parts = [], []
    for core in range(NCORES):
        no = res.results[core]["node_out"]                # [BL,ECH,128,N]
        node_parts.append(no.reshape(BL, D, N).transpose(0, 2, 1))
        wo = res.results[core]["w_out"]                   # [STEPS,BL,NSLAB,128,1]
        w_parts.append(wo.reshape(STEPS, BL, N).transpose(1, 0, 2))
    node_full = np.concatenate(node_parts, 0).astype(np.float32)
    w_full = np.concatenate(w_parts, 0).astype(np.float32)
    return node_full, w_full, res


def kernel(**inputs):
    node = np.asarray(inputs["node"], dtype=np.float32)
    node_mask = np.asarray(inputs["node_mask"])
    graphs = [np.asarray(inputs[k]) for k in
              ("arg_graph_1", "arg_graph_2", "arg_graph_3", "arg_graph_4",
               "punct_graph", "punct_graph_re")]
    params = {k: np.asarray(inputs[k]) for k in
              ("w_nw", "b_nw", "w_self", "b_self", "w_arg1", "w_arg2",
               "w_arg3", "w_arg4", "w_punct", "w_punct_re")}
    node_full, w_full, _ = run(node, node_mask, graphs, params, trace=False)
    return node_full, w_full


# revision 2
# speedup vs baseline: 1.0822x; 1.0822x over previous
"""ArgumentGCN message-passing kernel for TRN2, data-parallel over batch on 8 cores.

v3: mixed precision. The self/dw path (whose error hits the output directly)
stays bf16; the projection+aggregation path (whose error is damped by the
1/nn_num averaging) runs fp8 with DoubleRow (2 contraction rows per PE cell).

Per core (BL=4 local batches, N=512 nodes, D=256 feats, K=6 edge types, 2 steps),
all in transposed orientation (features/sources on partitions):

  dw[n]   = sigmoid(sum_d nodeT[d,n] * w_nw[d] + b_nw)              (bf16)
  P'[n,:] = (dw[n]/16) * sum_d nodeT_f8[d,n] * (16*W_k[d,e])        (fp8 DR)
  A[e,m]  = sum_{k,n} P'[n,k*256+e] * Gt[k][n,m]                    (fp8 DR)
  S[e,m]  = sum_d w_self[d,e] * nodeT[d,m]                          (bf16)
  new[e,m]= relu(A[e,m]*inv_nn[m] + S[e,m] + b_self[e])

W_k are host-scaled by 16 so ~0.02-magnitude entries stay fp8-normal; the 1/16
rides on the eviction scalar. Graphs are exact {0,1} fp8 (dd-masked, diag
zeroed on host); 1/nn_num is applied at eviction via a per-batch broadcast
tile built once with a K=1 matmul.
"""

import sys

for _p in ("/opt/trn_rl_repo",):
    if _p not in sys.path:
        sys.path.insert(0, _p)

import numpy as np
import ml_dtypes

import concourse.bass as bass
import concourse.mybir as mybir
import concourse.tile as tile
from concourse import bacc
from concourse.bass_utils import run_bass_kernel_spmd

B, N, D = 32, 512, 256
KEDGE = 6
STEPS = 2
NCORES = 8
BL = B // NCORES          # batches per core
NSLAB = N // 128          # 4 n-slabs
DSLAB = D // 128          # 2 d-slabs
ECH = D // 128            # 2 e-chunks
WECOLS = KEDGE * D        # 1536
PCHUNK = 512              # matmul free-dim / PSUM bank size (f32)
WSCALE = 16.0             # host pre-scale on edge weights (fp8-normal range)

F32 = mybir.dt.float32
BF16 = mybir.dt.bfloat16
FP8 = mybir.dt.float8e4
DR = mybir.MatmulPerfMode.DoubleRow
NPFP8 = ml_dtypes.float8_e4m3
NPBF16 = ml_dtypes.bfloat16


def build_nc(interleave=False, nall=True, psmain=5, psagg=3, ppbufs=6, tubufs=6, ev66=True, f8direct=False, warm=0):
    nc = bacc.Bacc("TRN2", target_bir_lowering=False, debug=False, num_devices=NCORES)

    # inputs (per-core shards; layouts chosen for DMA efficiency)
    gt = nc.dram_tensor("gt", [KEDGE, BL, 128, NSLAB, N], FP8, kind="ExternalInput")
    node0 = nc.dram_tensor("node0", [BL, DSLAB, 128, N], BF16, kind="ExternalInput")
    node0f = nc.dram_tensor("node0f", [BL, DSLAB, 128, N], FP8, kind="ExternalInput")
    we = nc.dram_tensor("we", [DSLAB, 128, WECOLS], FP8, kind="ExternalInput")
    ws = nc.dram_tensor("ws", [DSLAB, 128, D], BF16, kind="ExternalInput")
    wnw = nc.dram_tensor("wnw", [DSLAB, 128, 1], BF16, kind="ExternalInput")
    bself = nc.dram_tensor("bself", [ECH, 128, 1], F32, kind="ExternalInput")
    bnw = nc.dram_tensor("bnw", [128, 1], F32, kind="ExternalInput")
    invnn = nc.dram_tensor("invnn", [1, BL * N], F32, kind="ExternalInput")

    # outputs
    node_out = nc.dram_tensor("node_out", [BL, ECH, 128, N], F32, kind="ExternalOutput")
    w_out = nc.dram_tensor("w_out", [STEPS, BL, NSLAB, 128, 1], F32, kind="ExternalOutput")

    with tile.TileContext(nc) as tc:
        with (
            tc.tile_pool(name="consts", bufs=1) as consts,
            tc.tile_pool(name="gtp", bufs=KEDGE * BL) as gtp,
            tc.tile_pool(name="nodep", bufs=BL * STEPS) as nodep,
            tc.tile_pool(name="nodefp", bufs=BL * STEPS) as nodefp,
            tc.tile_pool(name="pprime", bufs=ppbufs) as pprime,
            tc.tile_pool(name="dwp", bufs=4) as dwp,
            tc.tile_pool(name="tup", bufs=tubufs) as tup,
            tc.tile_pool(name="foutp", bufs=2) as foutp,
            tc.tile_pool(name="ps_main", bufs=psmain, space="PSUM") as ps_main,
            tc.tile_pool(name="ps_agg", bufs=psagg, space="PSUM") as ps_agg,
        ):
            # ---- scalar HWDGE ring: we first, then batch-0 graphs ----
            we_sb = consts.tile([128, DSLAB, WECOLS], FP8)
            nc.scalar.dma_start(out=we_sb[:], in_=we.ap().rearrange("d p c -> p d c"))
            gt_sb = {}
            for k in range(KEDGE):
                t = gtp.tile([128, NSLAB, N], FP8, tag="gt", name="gt_sb")
                nc.scalar.dma_start(out=t[:], in_=gt.ap()[k, 0])
                gt_sb[(k, 0)] = t

            # ---- sync queue: invnn first (gates the DVE stream via invB), then
            # batch-0 state + weights so PE starts asap ----
            invrow = consts.tile([1, BL * N], F32)
            nc.sync.dma_start(out=invrow[:], in_=invnn.ap())
            node_sb, nodef_sb = {}, {}

            def load_node(b):
                t = nodep.tile([128, DSLAB, N], BF16, tag="node", name="node_in")
                nc.sync.dma_start(out=t[:], in_=node0.ap()[b].rearrange("d p m -> p d m"))
                node_sb[b] = t
                tf = nodefp.tile([128, DSLAB, N], FP8, tag="nodef", name="nodef_in")
                nc.sync.dma_start(out=tf[:], in_=node0f.ap()[b].rearrange("d p m -> p d m"))
                nodef_sb[b] = tf

            load_node(0)
            wnw_sb = consts.tile([128, DSLAB, 1], BF16)
            nc.sync.dma_start(out=wnw_sb[:], in_=wnw.ap().rearrange("d p c -> p d c"))
            bnw_sb = consts.tile([128, 1], F32)
            nc.sync.dma_start(out=bnw_sb[:], in_=bnw.ap())
            if nall:
                # batches 1..3 initial state in two consolidated DMAs
                nt = consts.tile([128, BL - 1, DSLAB, N], BF16, name="nall")
                nc.sync.dma_start(
                    out=nt[:], in_=node0.ap()[1:BL].rearrange("b d p m -> p b d m")
                )
                ntf = consts.tile([128, BL - 1, DSLAB, N], FP8, name="nallf")
                nc.sync.dma_start(
                    out=ntf[:], in_=node0f.ap()[1:BL].rearrange("b d p m -> p b d m")
                )
                for b in range(1, BL):
                    node_sb[b] = nt[:, b - 1]
                    nodef_sb[b] = ntf[:, b - 1]
            else:
                for b in range(1, BL):
                    load_node(b)
            ws_sb = consts.tile([128, DSLAB, D], BF16)
            nc.sync.dma_start(out=ws_sb[:], in_=ws.ap().rearrange("d p c -> p d c"))
            bself_sb = consts.tile([128, ECH, 1], F32)
            nc.sync.dma_start(out=bself_sb[:], in_=bself.ap().rearrange("c p o -> p c o"))
            ones_sb = consts.tile([1, 128], F32)
            nc.vector.memset(ones_sb[:], 1.0)

            # ---- per-batch 1/nn broadcast tiles (K=1 matmul broadcast);
            # early: the DVE copies gate all later DVE evictions ----
            invB = {}
            for b in range(BL):
                pib = ps_main.tile([128, PCHUNK], F32, tag="ps", name="ps_invb")
                # repeats are PE warm-up filler while input DMAs land: each
                # start=True matmul simply overwrites the bank, last one wins
                for _ in range((warm if b == 0 else 0) + 1):
                    nc.tensor.matmul(
                        pib[:], lhsT=ones_sb[:], rhs=invrow[:, b * N : (b + 1) * N],
                        start=True, stop=True,
                    )
                t = consts.tile([128, N], F32, tag="invb", name="invb", bufs=BL)
                nc.vector.tensor_copy(t[:], pib[:])
                invB[b] = t

            # ---- remaining graphs on the sync queue, batch-major ----
            for b in range(1, BL):
                for k in range(KEDGE):
                    t = gtp.tile([128, NSLAB, N], FP8, tag="gt", name="gt_sb")
                    nc.sync.dma_start(out=t[:], in_=gt.ap()[k, b])
                    gt_sb[(k, b)] = t

            # ---- iteration steps ----
            pp_sb, dwcs_sb = {}, {}

            def emit_dw_proj(step, b):
                    cur = node_sb[b]
                    curf = nodef_sb[b]

                    # (1) node-relatedness weights, column form per n-slab (bf16)
                    dwps = ps_main.tile([128, NSLAB], F32, tag="ps", name="ps_dw")
                    for s in range(NSLAB):
                        for d in range(DSLAB):
                            nc.tensor.matmul(
                                dwps[:, s : s + 1],
                                lhsT=cur[:, d, s * 128 : (s + 1) * 128],
                                rhs=wnw_sb[:, d, :],
                                start=(d == 0),
                                stop=(d == DSLAB - 1),
                            )
                    dwcol = dwp.tile([128, NSLAB], F32, tag="dwcol", name="dwcol")
                    nc.scalar.activation(
                        dwcol[:], dwps[:],
                        mybir.ActivationFunctionType.Sigmoid,
                        bias=bnw_sb[:],
                    )
                    for s in range(NSLAB):
                        nc.sync.dma_start(
                            out=w_out.ap()[step, b, s], in_=dwcol[:, s : s + 1]
                        )
                    dwcs = dwp.tile([128, NSLAB], F32, tag="dwcs", name="dwcs")
                    nc.vector.tensor_scalar_mul(dwcs[:], dwcol[:], 1.0 / WSCALE)
                    dwcs_sb[b] = dwcs

                    # (2) projections (fp8 DoubleRow over d), scaled at eviction
                    pp = pprime.tile([128, NSLAB, WECOLS], FP8, tag="pp", name="pp")
                    pp_sb[b] = pp
                    for s in range(NSLAB):
                        for c in range(WECOLS // PCHUNK):
                            pch = ps_main.tile([128, PCHUNK], F32, tag="ps", name="ps_proj")
                            nc.tensor.matmul(
                                pch[:],
                                lhsT=curf[:, 0:DSLAB, s * 128 : (s + 1) * 128],
                                rhs=we_sb[:, 0:DSLAB, c * PCHUNK : (c + 1) * PCHUNK],
                                start=True, stop=True, perf_mode=DR,
                            )
                            dst = pp[:, s, c * PCHUNK : (c + 1) * PCHUNK]
                            if ((s * 3 + c) % 2 == 0) if ev66 else (c % 2 == 0):
                                nc.vector.tensor_scalar_mul(dst, pch[:], dwcs[:, s : s + 1])
                            else:
                                nc.scalar.mul(dst, pch[:], dwcs[:, s : s + 1])

            def emit_agg(step, b):
                    last = step == STEPS - 1
                    cur = node_sb[b]
                    pp = pp_sb[b]
                    # (3) self (bf16) + aggregation (fp8 DR), inv_nn at eviction
                    if last:
                        out_t = foutp.tile([128, ECH, N], F32, tag="fout", name="fout")
                    else:
                        out_t = nodep.tile([128, DSLAB, N], BF16, tag="node", name="node_nx")
                        outf_t = nodefp.tile([128, DSLAB, N], FP8, tag="nodef", name="nodef_nx")
                    for c in range(ECH):
                        pa = ps_agg.tile([128, N], F32, tag="psagg", name="ps_agg")
                        ps_self = ps_agg.tile([128, N], F32, tag="psagg", name="ps_self")
                        for d in range(DSLAB):
                            nc.tensor.matmul(
                                ps_self[:],
                                lhsT=ws_sb[:, d, c * 128 : (c + 1) * 128],
                                rhs=cur[:, d, :],
                                start=(d == 0), stop=(d == DSLAB - 1),
                            )
                        for sp in range(NSLAB // 2):
                            for k in range(KEDGE):
                                nc.tensor.matmul(
                                    pa[:],
                                    lhsT=pp[:, 2 * sp : 2 * sp + 2,
                                            k * D + c * 128 : k * D + (c + 1) * 128],
                                    rhs=gt_sb[(k, b)][:, 2 * sp : 2 * sp + 2, :],
                                    start=(sp == 0 and k == 0),
                                    stop=(sp == NSLAB // 2 - 1 and k == KEDGE - 1),
                                    perf_mode=DR,
                                )
                        t_t = tup.tile([128, N], F32, tag="tu", name="t_t")
                        nc.vector.tensor_tensor(
                            out=t_t[:], in0=pa[:], in1=invB[b][:],
                            op=mybir.AluOpType.mult,
                        )
                        u_t = tup.tile([128, N], F32, tag="tu", name="u_t")
                        nc.vector.scalar_tensor_tensor(
                            out=u_t[:], in0=ps_self[:], scalar=1.0,
                            in1=t_t[:],
                            op0=mybir.AluOpType.mult, op1=mybir.AluOpType.add,
                        )
                        nc.scalar.activation(
                            out_t[:, c, :], u_t[:],
                            mybir.ActivationFunctionType.Relu,
                            bias=bself_sb[:, c, :],
                        )
                        if last:
                            nc.sync.dma_start(
                                out=node_out.ap()[b, c], in_=out_t[:, c, :]
                            )
                        elif f8direct:
                            nc.vector.tensor_scalar(
                                outf_t[:, c, :], u_t[:],
                                scalar1=bself_sb[:, c, :], scalar2=0.0,
                                op0=mybir.AluOpType.add, op1=mybir.AluOpType.max,
                            )
                        else:
                            nc.vector.tensor_copy(outf_t[:, c, :], out_t[:, c, :])
                    if last:
                        pass
                    else:
                        node_sb[b] = out_t
                        nodef_sb[b] = outf_t

            # step 0: front-load every batch's dw/proj before the first agg so
            # PE has work while the graphs stream in; then software-pipeline:
            # step-1 dw/proj for batch b emits right after step-0's agg of b.
            for b in range(BL):
                emit_dw_proj(0, b)
            if interleave:
                emit_agg(0, 0)
                emit_agg(0, 1)
                emit_dw_proj(1, 0)
                emit_agg(0, 2)
                emit_dw_proj(1, 1)
                emit_agg(0, 3)
                emit_dw_proj(1, 2)
                emit_agg(1, 0)
                emit_dw_proj(1, 3)
                emit_agg(1, 1)
                emit_agg(1, 2)
                emit_agg(1, 3)
            else:
                for b in range(BL):
                    emit_agg(0, b)
                for b in range(BL):
                    emit_dw_proj(1, b)
                    emit_agg(1, b)

    nc.compile()
    return nc


_NC_CACHE = None


def get_nc():
    global _NC_CACHE
    if _NC_CACHE is None:
        _NC_CACHE = build_nc()
    return _NC_CACHE


def _fp8(x):
    return np.clip(np.asarray(x, np.float32), -240.0, 240.0).astype(NPFP8)


def _prep_core_inputs(node, node_mask, graphs, params):
    """Host-side shard + layout prep. Returns in_maps list (one dict per core)."""
    f32 = np.float32
    mask = node_mask.astype(f32)                          # [B,N]
    dd = mask[:, None, :] * mask[:, :, None]              # [B,N,N]
    dd[:, np.arange(N), np.arange(N)] = 0.0

    G = np.stack(graphs, 0).astype(f32) * dd[None]        # [K,B,N,N]
    nn = G.sum(axis=(0, -1))                              # [B,N] dest counts
    nn = np.where(nn >= 1.0, nn, 1.0)
    inv_nn = (1.0 / nn).astype(f32)                       # [B,N]
    # transpose to [K,B,n,m], layout [K,B,part,slab,m] with n = slab*128+part
    Gt = np.ascontiguousarray(G.transpose(0, 1, 3, 2))
    Gt = Gt.reshape(KEDGE, B, NSLAB, 128, N).transpose(0, 1, 3, 2, 4)
    Gt = np.ascontiguousarray(Gt).astype(NPFP8)           # exact {0,1}

    nodeT = np.ascontiguousarray(node.transpose(0, 2, 1)) # [B,D,N]
    nodeT = nodeT.reshape(B, DSLAB, 128, N).astype(NPBF16)
    nodeTf = _fp8(nodeT.astype(f32))

    we = _fp8(WSCALE * np.concatenate(
        [params[k] for k in ("w_arg1", "w_arg2", "w_arg3", "w_arg4",
                             "w_punct", "w_punct_re")], axis=1
    ).reshape(DSLAB, 128, WECOLS))
    ws = params["w_self"].reshape(DSLAB, 128, D).astype(NPBF16)
    wnw = params["w_nw"].reshape(DSLAB, 128, 1).astype(NPBF16)
    bself = params["b_self"].astype(f32).reshape(ECH, 128, 1)
    bnw = np.full((128, 1), np.float32(params["b_nw"][0]), dtype=f32)

    in_maps = []
    for core in range(NCORES):
        sl = slice(core * BL, (core + 1) * BL)
        in_maps.append({
            "gt": np.ascontiguousarray(Gt[:, sl]),
            "node0": np.ascontiguousarray(nodeT[sl]),
            "node0f": np.ascontiguousarray(nodeTf[sl]),
            "invnn": np.ascontiguousarray(inv_nn[sl]).reshape(1, BL * N),
            "we": we, "ws": ws, "wnw": wnw, "bself": bself, "bnw": bnw,
        })
    return in_maps


def run(node, node_mask, graphs, params, trace=False, **spmd_kwargs):
    nc = get_nc()
    in_maps = _prep_core_inputs(node, node_mask, graphs, params)
    res = run_bass_kernel_spmd(
        nc, in_maps, core_ids=list(range(NCORES)), trace=trace, **spmd_kwargs
    )
    node_parts, w_parts = [], []
    for core in range(NCORES):
        no = res.results[core]["node_out"]                # [BL,ECH,128,N]
        node_parts.append(no.reshape(BL, D, N).transpose(0, 2, 1))
        wo = res.results[core]["w_out"]                   # [STEPS,BL,NSLAB,128,1]
        w_parts.append(wo.reshape(STEPS, BL, N).transpose(1, 0, 2))
    node_full = np.concatenate(node_parts, 0).astype(np.float32)
    w_full = np.concatenate(w_parts, 0).astype(np.float32)
    return node_full, w_full, res


def kernel(**inputs):
    node = np.asarray(inputs["node"], dtype=np.float32)
    node_mask = np.asarray(inputs["node_mask"])
    graphs = [np.asarray(inputs[k]) for k in
              ("arg_graph_1", "arg_graph_2", "arg_graph_3", "arg_graph_4",
               "punct_graph", "punct_graph_re")]
    params = {k: np.asarray(inputs[k]) for k in
              ("w_nw", "b_nw", "w_self", "b_self", "w_arg1", "w_arg2",
               "w_arg3", "w_arg4", "w_punct", "w_punct_re")}
    node_full, w_full, _ = run(node, node_mask, graphs, params, trace=False)
    return node_full, w_full


# revision 3
# speedup vs baseline: 1.2498x; 1.1549x over previous
"""ArgumentGCN message-passing kernel for TRN2, data-parallel over batch on 8 cores.

v3: mixed precision. The self/dw path (whose error hits the output directly)
stays bf16; the projection+aggregation path (whose error is damped by the
1/nn_num averaging) runs fp8 with DoubleRow (2 contraction rows per PE cell).

Per core (BL=4 local batches, N=512 nodes, D=256 feats, K=6 edge types, 2 steps),
all in transposed orientation (features/sources on partitions):

  dw[n]   = sigmoid(sum_d nodeT[d,n] * w_nw[d] + b_nw)              (bf16)
  P'[n,:] = (dw[n]/16) * sum_d nodeT_f8[d,n] * (16*W_k[d,e])        (fp8 DR)
  A[e,m]  = sum_{k,n} P'[n,k*256+e] * Gt[k][n,m]                    (fp8 DR)
  S[e,m]  = sum_d w_self[d,e] * nodeT[d,m]                          (bf16)
  new[e,m]= relu(A[e,m]*inv_nn[m] + S[e,m] + b_self[e])

W_k are host-scaled by 16 so ~0.02-magnitude entries stay fp8-normal; the 1/16
rides on the eviction scalar. Graphs are exact {0,1} fp8 (dd-masked, diag
zeroed on host); 1/nn_num is applied at eviction via a per-batch broadcast
tile built once with a K=1 matmul.
"""

import sys

for _p in ("/opt/trn_rl_repo",):
    if _p not in sys.path:
        sys.path.insert(0, _p)

import numpy as np
import ml_dtypes

import concourse.bass as bass
import concourse.mybir as mybir
import concourse.tile as tile
from concourse import bacc
from concourse.bass_utils import run_bass_kernel_spmd

B, N, D = 32, 512, 256
KEDGE = 6
STEPS = 2
NCORES = 8
BL = B // NCORES          # batches per core
NSLAB = N // 128          # 4 n-slabs
DSLAB = D // 128          # 2 d-slabs
ECH = D // 128            # 2 e-chunks
WECOLS = KEDGE * D        # 1536
PCHUNK = 512              # matmul free-dim / PSUM bank size (f32)
WSCALE = 16.0             # host pre-scale on edge weights (fp8-normal range)

F32 = mybir.dt.float32
BF16 = mybir.dt.bfloat16
FP8 = mybir.dt.float8e4
DR = mybir.MatmulPerfMode.DoubleRow
NPFP8 = ml_dtypes.float8_e4m3
NPBF16 = ml_dtypes.bfloat16


def build_nc(interleave=False, nall=True, psmain=5, psagg=3, ppbufs=6, tubufs=6, ev66=True, f8direct=False, warm=0, dwdr=False, tailsplit=True):
    nc = bacc.Bacc("TRN2", target_bir_lowering=False, debug=False, num_devices=NCORES)

    # inputs (per-core shards; layouts chosen for DMA efficiency)
    gt = nc.dram_tensor("gt", [KEDGE, BL, 128, NSLAB, N], FP8, kind="ExternalInput")
    node0 = nc.dram_tensor("node0", [BL, DSLAB, 128, N], BF16, kind="ExternalInput")
    node0f = nc.dram_tensor("node0f", [BL, DSLAB, 128, N], FP8, kind="ExternalInput")
    we = nc.dram_tensor("we", [DSLAB, 128, WECOLS], FP8, kind="ExternalInput")
    ws = nc.dram_tensor("ws", [DSLAB, 128, D], BF16, kind="ExternalInput")
    wnw = nc.dram_tensor("wnw", [DSLAB, 128, 1], BF16, kind="ExternalInput")
    wnwf = nc.dram_tensor("wnwf", [DSLAB, 128, 1], FP8, kind="ExternalInput")
    bself = nc.dram_tensor("bself", [ECH, 128, 1], F32, kind="ExternalInput")
    bnw = nc.dram_tensor("bnw", [128, 1], F32, kind="ExternalInput")
    invnn = nc.dram_tensor("invnn", [1, BL * N], F32, kind="ExternalInput")

    # outputs
    node_out = nc.dram_tensor("node_out", [BL, ECH, 128, N], F32, kind="ExternalOutput")
    w_out = nc.dram_tensor("w_out", [STEPS, BL, NSLAB, 128, 1], F32, kind="ExternalOutput")

    with tile.TileContext(nc) as tc:
        with (
            tc.tile_pool(name="consts", bufs=1) as consts,
            tc.tile_pool(name="gtp", bufs=KEDGE * BL) as gtp,
            tc.tile_pool(name="nodep", bufs=BL * STEPS) as nodep,
            tc.tile_pool(name="nodefp", bufs=BL * STEPS) as nodefp,
            tc.tile_pool(name="pprime", bufs=ppbufs) as pprime,
            tc.tile_pool(name="dwp", bufs=4) as dwp,
            tc.tile_pool(name="tup", bufs=tubufs) as tup,
            tc.tile_pool(name="foutp", bufs=2) as foutp,
            tc.tile_pool(name="ps_main", bufs=psmain, space="PSUM") as ps_main,
            tc.tile_pool(name="ps_agg", bufs=psagg, space="PSUM") as ps_agg,
        ):
            # ---- scalar HWDGE ring: we first, then batch-0 graphs ----
            we_sb = consts.tile([128, DSLAB, WECOLS], FP8)
            nc.scalar.dma_start(out=we_sb[:], in_=we.ap().rearrange("d p c -> p d c"))
            gt_sb = {}
            for k in range(KEDGE):
                t = gtp.tile([128, NSLAB, N], FP8, tag="gt", name="gt_sb")
                nc.scalar.dma_start(out=t[:], in_=gt.ap()[k, 0])
                gt_sb[(k, 0)] = t

            # ---- sync queue: invnn first (gates the DVE stream via invB), then
            # batch-0 state + weights so PE starts asap ----
            invrow = consts.tile([1, BL * N], F32)
            nc.sync.dma_start(out=invrow[:], in_=invnn.ap())
            node_sb, nodef_sb = {}, {}

            def load_node(b):
                t = nodep.tile([128, DSLAB, N], BF16, tag="node", name="node_in")
                nc.sync.dma_start(out=t[:], in_=node0.ap()[b].rearrange("d p m -> p d m"))
                node_sb[b] = t
                tf = nodefp.tile([128, DSLAB, N], FP8, tag="nodef", name="nodef_in")
                nc.sync.dma_start(out=tf[:], in_=node0f.ap()[b].rearrange("d p m -> p d m"))
                nodef_sb[b] = tf

            load_node(0)
            wnw_sb = consts.tile([128, DSLAB, 1], BF16)
            nc.sync.dma_start(out=wnw_sb[:], in_=wnw.ap().rearrange("d p c -> p d c"))
            wnwf_sb = consts.tile([128, DSLAB, 1], FP8)
            nc.sync.dma_start(out=wnwf_sb[:], in_=wnwf.ap().rearrange("d p c -> p d c"))
            bnw_sb = consts.tile([128, 1], F32)
            nc.sync.dma_start(out=bnw_sb[:], in_=bnw.ap())
            if nall:
                # batches 1..3 initial state in two consolidated DMAs
                nt = consts.tile([128, BL - 1, DSLAB, N], BF16, name="nall")
                nc.sync.dma_start(
                    out=nt[:], in_=node0.ap()[1:BL].rearrange("b d p m -> p b d m")
                )
                ntf = consts.tile([128, BL - 1, DSLAB, N], FP8, name="nallf")
                nc.sync.dma_start(
                    out=ntf[:], in_=node0f.ap()[1:BL].rearrange("b d p m -> p b d m")
                )
                for b in range(1, BL):
                    node_sb[b] = nt[:, b - 1]
                    nodef_sb[b] = ntf[:, b - 1]
            else:
                for b in range(1, BL):
                    load_node(b)
            ws_sb = consts.tile([128, DSLAB, D], BF16)
            nc.sync.dma_start(out=ws_sb[:], in_=ws.ap().rearrange("d p c -> p d c"))
            bself_sb = consts.tile([128, ECH, 1], F32)
            nc.sync.dma_start(out=bself_sb[:], in_=bself.ap().rearrange("c p o -> p c o"))
            ones_sb = consts.tile([1, 128], F32)
            nc.vector.memset(ones_sb[:], 1.0)

            # ---- per-batch 1/nn broadcast tiles (K=1 matmul broadcast);
            # early: the DVE copies gate all later DVE evictions ----
            invB = {}
            for b in range(BL):
                pib = ps_main.tile([128, PCHUNK], F32, tag="ps", name="ps_invb")
                # repeats are PE warm-up filler while input DMAs land: each
                # start=True matmul simply overwrites the bank, last one wins
                for _ in range((warm if b == 0 else 0) + 1):
                    nc.tensor.matmul(
                        pib[:], lhsT=ones_sb[:], rhs=invrow[:, b * N : (b + 1) * N],
                        start=True, stop=True,
                    )
                t = consts.tile([128, N], F32, tag="invb", name="invb", bufs=BL)
                nc.vector.tensor_copy(t[:], pib[:])
                invB[b] = t

            # ---- remaining graphs on the sync queue, batch-major ----
            for b in range(1, BL):
                for k in range(KEDGE):
                    t = gtp.tile([128, NSLAB, N], FP8, tag="gt", name="gt_sb")
                    nc.sync.dma_start(out=t[:], in_=gt.ap()[k, b])
                    gt_sb[(k, b)] = t

            # ---- iteration steps ----
            pp_sb, dwcs_sb = {}, {}

            def emit_dw_proj(step, b):
                    cur = node_sb[b]
                    curf = nodef_sb[b]

                    # (1) node-relatedness weights, column form per n-slab (bf16)
                    dwps = ps_main.tile([128, NSLAB], F32, tag="ps", name="ps_dw")
                    if dwdr:
                        for s in range(NSLAB):
                            nc.tensor.matmul(
                                dwps[:, s : s + 1],
                                lhsT=curf[:, 0:DSLAB, s * 128 : (s + 1) * 128],
                                rhs=wnwf_sb[:, 0:DSLAB, :],
                                start=True, stop=True, perf_mode=DR,
                            )
                    else:
                        for s in range(NSLAB):
                            for d in range(DSLAB):
                                nc.tensor.matmul(
                                    dwps[:, s : s + 1],
                                    lhsT=cur[:, d, s * 128 : (s + 1) * 128],
                                    rhs=wnw_sb[:, d, :],
                                    start=(d == 0),
                                    stop=(d == DSLAB - 1),
                                )
                    dwcol = dwp.tile([128, NSLAB], F32, tag="dwcol", name="dwcol")
                    nc.scalar.activation(
                        dwcol[:], dwps[:],
                        mybir.ActivationFunctionType.Sigmoid,
                        bias=bnw_sb[:],
                        scale=(1.0 / WSCALE) if dwdr else 1.0,
                    )
                    for s in range(NSLAB):
                        nc.sync.dma_start(
                            out=w_out.ap()[step, b, s], in_=dwcol[:, s : s + 1]
                        )
                    dwcs = dwp.tile([128, NSLAB], F32, tag="dwcs", name="dwcs")
                    nc.vector.tensor_scalar_mul(dwcs[:], dwcol[:], 1.0 / WSCALE)
                    dwcs_sb[b] = dwcs

                    # (2) projections (fp8 DoubleRow over d), scaled at eviction
                    pp = pprime.tile([128, NSLAB, WECOLS], FP8, tag="pp", name="pp")
                    pp_sb[b] = pp
                    for s in range(NSLAB):
                        for c in range(WECOLS // PCHUNK):
                            pch = ps_main.tile([128, PCHUNK], F32, tag="ps", name="ps_proj")
                            nc.tensor.matmul(
                                pch[:],
                                lhsT=curf[:, 0:DSLAB, s * 128 : (s + 1) * 128],
                                rhs=we_sb[:, 0:DSLAB, c * PCHUNK : (c + 1) * PCHUNK],
                                start=True, stop=True, perf_mode=DR,
                            )
                            dst = pp[:, s, c * PCHUNK : (c + 1) * PCHUNK]
                            if ((s * 3 + c) % 2 == 0) if ev66 else (c % 2 == 0):
                                nc.vector.tensor_scalar_mul(dst, pch[:], dwcs[:, s : s + 1])
                            else:
                                nc.scalar.mul(dst, pch[:], dwcs[:, s : s + 1])

            def emit_agg(step, b):
                    last = step == STEPS - 1
                    cur = node_sb[b]
                    pp = pp_sb[b]
                    # (3) self (bf16) + aggregation (fp8 DR), inv_nn at eviction
                    if last:
                        out_t = foutp.tile([128, ECH, N], F32, tag="fout", name="fout")
                    else:
                        out_t = nodep.tile([128, DSLAB, N], BF16, tag="node", name="node_nx")
                        outf_t = nodefp.tile([128, DSLAB, N], FP8, tag="nodef", name="nodef_nx")
                    for c in range(ECH):
                        # the very last aggregation is split into two m-halves
                        # (separate PSUM banks) so its eviction chain overlaps
                        # the second half's matmuls instead of trailing the kernel
                        split = tailsplit and last and b == BL - 1 and c == ECH - 1
                        halves = [(0, N)] if not split else [(0, N // 2), (N // 2, N)]
                        ps_self = ps_agg.tile([128, N], F32, tag="psagg", name="ps_self")
                        for d in range(DSLAB):
                            nc.tensor.matmul(
                                ps_self[:],
                                lhsT=ws_sb[:, d, c * 128 : (c + 1) * 128],
                                rhs=cur[:, d, :],
                                start=(d == 0), stop=(d == DSLAB - 1),
                            )
                        for m0, m1 in halves:
                            pa = ps_agg.tile([128, m1 - m0], F32, tag="psagg", name="ps_agg")
                            for sp in range(NSLAB // 2):
                                for k in range(KEDGE):
                                    nc.tensor.matmul(
                                        pa[:],
                                        lhsT=pp[:, 2 * sp : 2 * sp + 2,
                                                k * D + c * 128 : k * D + (c + 1) * 128],
                                        rhs=gt_sb[(k, b)][:, 2 * sp : 2 * sp + 2, m0:m1],
                                        start=(sp == 0 and k == 0),
                                        stop=(sp == NSLAB // 2 - 1 and k == KEDGE - 1),
                                        perf_mode=DR,
                                    )
                            t_t = tup.tile([128, m1 - m0], F32, tag="tu", name="t_t")
                            nc.vector.tensor_tensor(
                                out=t_t[:], in0=pa[:], in1=invB[b][:, m0:m1],
                                op=mybir.AluOpType.mult,
                            )
                            u_t = tup.tile([128, m1 - m0], F32, tag="tu", name="u_t")
                            nc.vector.scalar_tensor_tensor(
                                out=u_t[:], in0=ps_self[:, m0:m1], scalar=1.0,
                                in1=t_t[:],
                                op0=mybir.AluOpType.mult, op1=mybir.AluOpType.add,
                            )
                            nc.scalar.activation(
                                out_t[:, c, m0:m1], u_t[:],
                                mybir.ActivationFunctionType.Relu,
                                bias=bself_sb[:, c, :],
                            )
                            if last:
                                nc.sync.dma_start(
                                    out=node_out.ap()[b, c, :, m0:m1],
                                    in_=out_t[:, c, m0:m1],
                                )
                            elif f8direct:
                                nc.vector.tensor_scalar(
                                    outf_t[:, c, :], u_t[:],
                                    scalar1=bself_sb[:, c, :], scalar2=0.0,
                                    op0=mybir.AluOpType.add, op1=mybir.AluOpType.max,
                                )
                            else:
                                nc.vector.tensor_copy(outf_t[:, c, m0:m1], out_t[:, c, m0:m1])
                    if last:
                        pass
                    else:
                        node_sb[b] = out_t
                        nodef_sb[b] = outf_t

            # step 0: front-load every batch's dw/proj before the first agg so
            # PE has work while the graphs stream in; then software-pipeline:
            # step-1 dw/proj for batch b emits right after step-0's agg of b.
            for b in range(BL):
                emit_dw_proj(0, b)
            if interleave:
                emit_agg(0, 0)
                emit_agg(0, 1)
                emit_dw_proj(1, 0)
                emit_agg(0, 2)
                emit_dw_proj(1, 1)
                emit_agg(0, 3)
                emit_dw_proj(1, 2)
                emit_agg(1, 0)
                emit_dw_proj(1, 3)
                emit_agg(1, 1)
                emit_agg(1, 2)
                emit_agg(1, 3)
            else:
                for b in range(BL):
                    emit_agg(0, b)
                for b in range(BL):
                    emit_dw_proj(1, b)
                    emit_agg(1, b)

    nc.compile()
    return nc


_NC_CACHE = None


def get_nc():
    global _NC_CACHE
    if _NC_CACHE is None:
        _NC_CACHE = build_nc()
    return _NC_CACHE


def _fp8(x):
    return np.clip(np.asarray(x, np.float32), -240.0, 240.0).astype(NPFP8)


def _prep_core_inputs(node, node_mask, graphs, params):
    """Host-side shard + layout prep. Returns in_maps list (one dict per core)."""
    f32 = np.float32
    mask = node_mask.astype(f32)                          # [B,N]
    dd = mask[:, None, :] * mask[:, :, None]              # [B,N,N]
    dd[:, np.arange(N), np.arange(N)] = 0.0

    G = np.stack(graphs, 0).astype(f32) * dd[None]        # [K,B,N,N]
    nn = G.sum(axis=(0, -1))                              # [B,N] dest counts
    nn = np.where(nn >= 1.0, nn, 1.0)
    inv_nn = (1.0 / nn).astype(f32)                       # [B,N]
    # transpose to [K,B,n,m], layout [K,B,part,slab,m] with n = slab*128+part
    Gt = np.ascontiguousarray(G.transpose(0, 1, 3, 2))
    Gt = Gt.reshape(KEDGE, B, NSLAB, 128, N).transpose(0, 1, 3, 2, 4)
    Gt = np.ascontiguousarray(Gt).astype(NPFP8)           # exact {0,1}

    nodeT = np.ascontiguousarray(node.transpose(0, 2, 1)) # [B,D,N]
    nodeT = nodeT.reshape(B, DSLAB, 128, N).astype(NPBF16)
    nodeTf = _fp8(nodeT.astype(f32))

    we = _fp8(WSCALE * np.concatenate(
        [params[k] for k in ("w_arg1", "w_arg2", "w_arg3", "w_arg4",
                             "w_punct", "w_punct_re")], axis=1
    ).reshape(DSLAB, 128, WECOLS))
    ws = params["w_self"].reshape(DSLAB, 128, D).astype(NPBF16)
    wnw = params["w_nw"].reshape(DSLAB, 128, 1).astype(NPBF16)
    wnwf = _fp8(WSCALE * params["w_nw"].reshape(DSLAB, 128, 1))
    bself = params["b_self"].astype(f32).reshape(ECH, 128, 1)
    bnw = np.full((128, 1), np.float32(params["b_nw"][0]), dtype=f32)

    in_maps = []
    for core in range(NCORES):
        sl = slice(core * BL, (core + 1) * BL)
        in_maps.append({
            "gt": np.ascontiguousarray(Gt[:, sl]),
            "node0": np.ascontiguousarray(nodeT[sl]),
            "node0f": np.ascontiguousarray(nodeTf[sl]),
            "invnn": np.ascontiguousarray(inv_nn[sl]).reshape(1, BL * N),
            "we": we, "ws": ws, "wnw": wnw, "wnwf": wnwf,
            "bself": bself, "bnw": bnw,
        })
    return in_maps


def run(node, node_mask, graphs, params, trace=False, **spmd_kwargs):
    nc = get_nc()
    in_maps = _prep_core_inputs(node, node_mask, graphs, params)
    res = run_bass_kernel_spmd(
        nc, in_maps, core_ids=list(range(NCORES)), trace=trace, **spmd_kwargs
    )
    node_parts, w_parts = [], []
    for core in range(NCORES):
        no = res.results[core]["node_out"]                # [BL,ECH,128,N]
        node_parts.append(no.reshape(BL, D, N).transpose(0, 2, 1))
        wo = res.results[core]["w_out"]                   # [STEPS,BL,NSLAB,128,1]
        w_parts.append(wo.reshape(STEPS, BL, N).transpose(1, 0, 2))
    node_full = np.concatenate(node_parts, 0).astype(np.float32)
    w_full = np.concatenate(w_parts, 0).astype(np.float32)
    return node_full, w_full, res


def kernel(**inputs):
    node = np.asarray(inputs["node"], dtype=np.float32)
    node_mask = np.asarray(inputs["node_mask"])
    graphs = [np.asarray(inputs[k]) for k in
              ("arg_graph_1", "arg_graph_2", "arg_graph_3", "arg_graph_4",
               "punct_graph", "punct_graph_re")]
    params = {k: np.asarray(inputs[k]) for k in
              ("w_nw", "b_nw", "w_self", "b_self", "w_arg1", "w_arg2",
               "w_arg3", "w_arg4", "w_punct", "w_punct_re")}
    node_full, w_full, _ = run(node, node_mask, graphs, params, trace=False)
    return node_full, w_full
